# revision 1
# baseline (speedup 1.0000x reference)
"""Distributed mean-squared-distance kernel for Trainium2 (8 NeuronCores).

Computes  out[b] = mean_n ||x[b] - features[n]||^2  for x:[1024,128],
features:[100000,128].

Because the mean is linear, the full [B, N] distance matrix is never needed:

    out[b] = ||x_b||^2 + (1/N) * sum_n ||f_n||^2 - (2/N) * x_b . (sum_n f_n)

Each core streams a 1/8 shard of `features` once (memory-bound roofline),
producing the shard's partial scalar S2 = sum ||f_n||^2 and partial vector
S1 = sum f_n, then combines them with the (replicated) x into a partial
output y_c[b] = x2[b]/8 + S2_c/N - (2/N) x_b . S1_c.  The host gather step
sums the 8 partial outputs (the all-reduce of the sharding hint).

Engine split per feature tile: HWDGE DMA streams, DVE reduces over the
row-chunk axis (per-d partial sums), ACT squares with free-dim accumulation
(sum of squares).  Cross-partition sums + broadcast use GPSIMD
partition_all_reduce; the tail combine is plain DVE/ACT ops.
"""

import sys

sys.path.insert(0, "/opt/trn_rl_repo")

import numpy as np

import concourse.bacc as bacc
import concourse.tile as tile
from concourse import mybir
from concourse import bass_isa
from concourse import bass_utils

P = 128                    # SBUF partitions
B, D, N = 1024, 128, 100000
NCORES = 8
TPP = 98                   # feature rows per partition per core
RPC = P * TPP              # 12544 feature rows per core (padded shard)
PAD_N = RPC * NCORES       # 100352 rows after zero-padding
NT = 7                     # feature mega-tiles per core
TT = TPP // NT             # 14 rows per partition per mega-tile
BT = B // P                # 8 x-rows per partition

F32 = mybir.dt.float32
AX = mybir.AxisListType
OP = mybir.AluOpType
AF = mybir.ActivationFunctionType


def _build():
    nc = bacc.Bacc("TRN2", debug=False, num_devices=NCORES)
    f_d = nc.dram_tensor("features", [RPC, D], F32, kind="ExternalInput").ap()
    x_d = nc.dram_tensor("x", [B, D], F32, kind="ExternalInput").ap()
    y_d = nc.dram_tensor("y", [P, BT], F32, kind="ExternalOutput").ap()

    # Row r of the shard maps to partition r // TPP, chunk r % TPP: each
    # partition reads one contiguous (TPP*D*4 B) run of DRAM per core.
    f_view = f_d.rearrange("(p t) d -> p t d", p=P)    # [128, 98, 128]
    x_view = x_d.rearrange("(p t) d -> p t d", p=P)    # [128, 8, 128]

    with tile.TileContext(nc) as tc:
        with (
            # Distinct tags below give every feature tile its own slot, so no
            # load DMA ever waits on a compute semaphore (HWDGE DMA
            # descriptors only support a single sync-wait command).
            tc.tile_pool(name="fpool", bufs=1) as fpool,
            tc.tile_pool(name="scratch", bufs=1) as scratch,
            tc.tile_pool(name="small", bufs=1) as small,
        ):
            # x path: replicated x; row sums of squares done on DVE below.
            xt = small.tile([P, BT, D], F32)
            nc.sync.dma_start(out=xt, in_=x_view)

            # Feature stream.  ACT squares each tile (free-dim accumulate
            # gives the per-partition sum of squares); DVE folds the 7 tiles
            # with a tree of contiguous adds (2 input elems/cycle) and then
            # does a single strided per-d reduce, which is ~2x cheaper than
            # strided-reducing every tile.
            acc2 = small.tile([P, NT], F32)
            fsq = scratch.tile([P, TT * D], F32)
            fts = []
            for i in range(NT):
                ft = fpool.tile([P, TT, D], F32, tag=f"ft{i}")
                fts.append(ft)
                nc.sync.dma_start(out=ft, in_=f_view[:, i * TT : (i + 1) * TT, :])
                nc.scalar.activation(
                    out=fsq, in_=ft.rearrange("p t d -> p (t d)"), func=AF.Square,
                    accum_out=acc2[:, i : i + 1],
                )

            g0 = small.tile([P, TT, D], F32)
            nc.vector.tensor_add(g0, fts[0], fts[1])
            g1 = small.tile([P, TT, D], F32)
            nc.vector.tensor_add(g1, fts[2], fts[3])
            g2 = small.tile([P, TT, D], F32)
            nc.vector.tensor_add(g2, fts[4], fts[5])
            h0 = small.tile([P, TT, D], F32)
            nc.vector.tensor_add(h0, g0, g1)
            h1 = small.tile([P, TT, D], F32)
            nc.vector.tensor_add(h1, g2, fts[6])
            ht = small.tile([P, TT, D], F32)
            nc.vector.tensor_add(ht, h0, h1)

            hp = small.tile([P, NT, D], F32)
            nc.vector.tensor_add(hp, ht[:, : NT, :], ht[:, NT:, :])
            s1_pre = small.tile([P, D], F32)
            nc.vector.tensor_reduce(
                out=s1_pre, in_=hp.rearrange("p t d -> p d t"),
                axis=AX.X, op=OP.add,
            )
            s2_col = small.tile([P, 1], F32)
            nc.vector.tensor_reduce(out=s2_col, in_=acc2, axis=AX.X, op=OP.add)

            # x2 on DVE: one big multiply + one contiguous-inner reduce.
            xx = small.tile([P, BT, D], F32)
            nc.vector.tensor_mul(out=xx, in0=xt, in1=xt)
            x2cols = small.tile([P, BT], F32)
            nc.vector.tensor_reduce(out=x2cols, in_=xx, axis=AX.X, op=OP.add)

            # Cross-partition all-reduce (result replicated to every
            # partition, which is exactly the broadcast the combine needs).
            s1b = small.tile([P, D], F32)
            nc.gpsimd.partition_all_reduce(
                s1b, s1_pre, channels=P, reduce_op=bass_isa.ReduceOp.add
            )
            s2b = small.tile([P, 1], F32)
            nc.gpsimd.partition_all_reduce(
                s2b, s2_col, channels=P, reduce_op=bass_isa.ReduceOp.add
            )
            s2n = small.tile([P, 1], F32)
            nc.scalar.mul(s2n, s2b, 1.0 / N)

            # dot_j[p] = x[p*8+j] . S1: one multiply against S1 broadcast
            # across the 8 row-blocks via a stride-0 middle AP dim.
            import concourse.bass as bass
            s1b_rep = bass.AP(
                tensor=s1b.tensor, offset=s1b.offset,
                ap=[list(s1b.ap[0]), [0, BT], list(s1b.ap[1])],
            )
            xprod = small.tile([P, BT, D], F32)
            nc.vector.tensor_mul(out=xprod, in0=xt, in1=s1b_rep)
            dot8 = small.tile([P, BT], F32)
            nc.vector.tensor_reduce(out=dot8, in_=xprod, axis=AX.X, op=OP.add)

            # y = 0.125*x2 + (S2/N - (2/N)*dot)
            dotb = small.tile([P, BT], F32)
            nc.scalar.activation(
                out=dotb, in_=dot8, func=AF.Identity, bias=s2n, scale=-2.0 / N,
            )
            x2s = small.tile([P, BT], F32)
            nc.scalar.mul(x2s, x2cols, 1.0 / NCORES)
            y_all = small.tile([P, BT], F32)
            nc.vector.tensor_add(y_all, dotb, x2s)
            # (tensor_add reads dotb/x2s which are tiny; keep on DVE)
            nc.sync.dma_start(out=y_d, in_=y_all)
    nc.compile()
    return nc


_nc_cache = None


def _get_nc():
    global _nc_cache
    if _nc_cache is None:
        _nc_cache = _build()
    return _nc_cache


def make_in_maps(x: np.ndarray, features: np.ndarray) -> list[dict[str, np.ndarray]]:
    x = np.ascontiguousarray(x, dtype=np.float32)
    features = np.ascontiguousarray(features, dtype=np.float32)
    padded = np.zeros((PAD_N, D), dtype=np.float32)
    padded[: features.shape[0]] = features
    return [
        {"features": padded[c * RPC : (c + 1) * RPC], "x": x}
        for c in range(NCORES)
    ]


def kernel(x: np.ndarray, features: np.ndarray, _trace: bool = False):
    nc = _get_nc()
    in_maps = make_in_maps(x, features)
    res = bass_utils.run_bass_kernel_spmd(
        nc, in_maps, core_ids=list(range(NCORES)), trace=_trace
    )
    out = np.zeros(B, dtype=np.float64)
    for c in range(NCORES):
        # y[p, t] holds output row p*BT + t, so row-major reshape is exact.
        out += res.results[c]["y"].reshape(B).astype(np.float64)
    out = out.astype(np.float32)
    if _trace:
        return out, res
    return out

